# revision 1
# baseline (speedup 1.0000x reference)
"""MoE MLP (E=4, top-2 routing) Trainium2 kernel, 8 NeuronCores.

Strategy ("pair-group" sharding): tokens are grouped on the host by their
routed expert PAIR (6 possible pairs for E=4).  Each of the 8 cores gets one
contiguous window of tokens that all share the same expert pair (a, b), plus
the full weights of those two experts.  Each core computes
    z = p_a * gelu(x @ w1[a]) @ w2[a] + p_b * gelu(x @ w1[b]) @ w2[b] + res
for its window — entirely locally, so no collectives are needed.  The host
only permutes rows back to token order afterwards (no arithmetic on the
common path).

Tokens with !=2 routed experts are decomposed into "virtual rows" of <=2
contributions each; if the resulting group structure does not fit 8 windows
(non-top-2 routing), a dense fallback (every core: 256 tokens x all 4
experts) is used.
"""
import math
import sys

import numpy as np

try:
    import concourse.bass as bass  # noqa: F401
except Exception:
    sys.path.insert(0, "/opt/trn_rl_repo")

import concourse.bacc as bacc
import concourse.bass as bass
import concourse.mybir as mybir
import concourse.tile as tile
from concourse.bass_utils import run_bass_kernel_spmd

S, B, H, F, E = 1024, 2, 1024, 4096, 4
T = S * B
N_CORES = 8
NH = H // 128   # 8
NF = F // 128   # 32
MM_DT = mybir.dt.float16  # full PE rate, ~2^-11 operand rounding
MM_NP = np.float16


def _plan_windows(routing_map, probs):
    """Decompose tokens into virtual rows and pack them into 8 pure windows.

    Returns (n_slots, C, windows) where windows is a list of 8 tuples
    (experts_tuple, vrow_list); each vrow is (t, pa, pb, first).
    """
    groups = {}
    for t in range(T):
        es = np.nonzero(routing_map[t])[0]
        if len(es) == 0:
            groups.setdefault((0, 0), []).append((t, 0.0, 0.0, True))
        else:
            for k in range(0, len(es), 2):
                pair = es[k : k + 2]
                if len(pair) == 1:
                    a = b = int(pair[0])
                    pa, pb = float(probs[t, a]), 0.0
                else:
                    a, b = int(pair[0]), int(pair[1])
                    pa, pb = float(probs[t, a]), float(probs[t, b])
                groups.setdefault((a, b), []).append((t, pa, pb, k == 0))

    for C in (128, 256, 384, 512):
        if sum(math.ceil(len(g) / C) for g in groups.values()) <= N_CORES:
            windows = []
            for (a, b), lst in sorted(groups.items()):
                nparts = math.ceil(len(lst) / C)
                step = math.ceil(len(lst) / nparts)
                for i in range(nparts):
                    windows.append(((a, b), lst[i * step : (i + 1) * step]))
            while len(windows) < N_CORES:
                windows.append(((0, 0), []))
            return 2, C, windows
    # dense fallback: all 4 experts on every core, 256 tokens per core
    C = T // N_CORES
    windows = []
    for c in range(N_CORES):
        lst = [(t, 0.0, 0.0, True) for t in range(c * C, (c + 1) * C)]
        windows.append(((0, 1, 2, 3), lst))
    return E, C, windows


_NC_CACHE = {}


def _build_nc(n_slots, C):
    key = (n_slots, C)
    if key in _NC_CACHE:
        return _NC_CACHE[key]
    NT = C // 128
    f32 = mybir.dt.float32
    nc = bacc.Bacc("TRN2", target_bir_lowering=False, debug=False,
                   num_devices=N_CORES)
    xt_d = nc.declare_dram_parameter("xt", [H, C], MM_DT, isOutput=False)
    w1_d = nc.declare_dram_parameter("w1b", [n_slots, NF, 128, H], MM_DT,
                                     isOutput=False)
    w2_d = nc.declare_dram_parameter("w2b", [n_slots, F, H], MM_DT,
                                     isOutput=False)
    pp_d = nc.declare_dram_parameter("pp", [n_slots, C], f32, isOutput=False)
    res_d = nc.declare_dram_parameter("res", [C, H], f32, isOutput=False)
    out_d = nc.declare_dram_parameter("out", [C, H], f32, isOutput=True)

    with tile.TileContext(nc) as tc:
        with (
            tc.tile_pool(name="resident", bufs=1) as rpool,
            tc.tile_pool(name="w1", bufs=8) as w1pool,
            tc.tile_pool(name="w2", bufs=12) as w2pool,
            tc.tile_pool(name="abig", bufs=2) as apool,
            tc.tile_pool(name="tmp", bufs=4) as tpool,
            tc.tile_pool(name="pa", bufs=3, space="PSUM") as papool,
            tc.tile_pool(name="py", bufs=NT, space="PSUM") as pypool,
        ):
            xt_sb = rpool.tile([128, NH, C], MM_DT, tag="xt")
            nc.sync.dma_start(
                xt_sb[:], xt_d.ap().rearrange("(hc h) c -> h hc c", h=128))
            res_sb = rpool.tile([128, NT, H], f32, tag="res")
            nc.sync.dma_start(
                res_sb[:], res_d.ap().rearrange("(tc t) d -> t tc d", t=128))
            pp_sb = rpool.tile([128, n_slots, NT], f32, tag="pp")
            nc.sync.dma_start(
                pp_sb[:], pp_d.ap().rearrange("s (tc t) -> t s tc", t=128))
            z_sb = rpool.tile([128, NT, H], f32, tag="z")

            for s in range(n_slots):
                a_big = apool.tile([128, NF, C], MM_DT, tag="a")
                for Fc in range(NF):
                    w1t = w1pool.tile([128, H], MM_DT, tag="w1")
                    nc.sync.dma_start(w1t[:], w1_d[s, Fc])
                    pa = papool.tile([128, C], f32, tag="pa")
                    for Hc in range(NH):
                        nc.tensor.matmul(
                            pa[:, :],
                            w1t[:, Hc * 128:(Hc + 1) * 128],
                            xt_sb[:, Hc, :],
                            start=(Hc == 0), stop=(Hc == NH - 1))
                    nc.scalar.activation(
                        a_big[:, Fc, :], pa[:, :],
                        mybir.ActivationFunctionType.Gelu)
                for Hh in range(2):
                    psum_ys = [pypool.tile([128, 512], f32, tag="py",
                                           name=f"py_{s}_{Hh}_{i}")
                               for i in range(NT)]
                    for Fc in range(NF):
                        w2t = w2pool.tile([128, 512], MM_DT, tag="w2")
                        nc.sync.dma_start(
                            w2t[:],
                            w2_d[s, Fc * 128:(Fc + 1) * 128,
                                 Hh * 512:(Hh + 1) * 512])
                        for Tc in range(NT):
                            nc.tensor.matmul(
                                psum_ys[Tc][:, :],
                                a_big[:, Fc,
                                      Tc * 128:(Tc + 1) * 128],
                                w2t[:, :],
                                start=(Fc == 0), stop=(Fc == NF - 1))
                    for Tc in range(NT):
                        zsl = z_sb[:, Tc, Hh * 512:(Hh + 1) * 512]
                        pcol = pp_sb[:, s, Tc:Tc + 1]
                        if s == 0:
                            nc.vector.tensor_scalar(
                                zsl, psum_ys[Tc][:, :], pcol, None,
                                mybir.AluOpType.mult)
                            nc.vector.tensor_add(
                                zsl, zsl,
                                res_sb[:, Tc, Hh * 512:(Hh + 1) * 512])
                        else:
                            tmp = tpool.tile([128, 512], f32, tag="tmp")
                            nc.vector.tensor_scalar(
                                tmp[:], psum_ys[Tc][:, :], pcol, None,
                                mybir.AluOpType.mult)
                            nc.vector.tensor_add(zsl, zsl, tmp[:])
            nc.sync.dma_start(
                out_d.ap().rearrange("(tc t) d -> t tc d", t=128), z_sb[:])
    nc.compile()
    _NC_CACHE[key] = nc
    return nc


def kernel(hidden_states, mlp_residual, probs, routing_map, w1, w2,
           _trace=False):
    hidden_states = np.ascontiguousarray(np.asarray(hidden_states, np.float32))
    mlp_residual = np.ascontiguousarray(np.asarray(mlp_residual, np.float32))
    probs = np.asarray(probs, np.float32)
    routing_map = np.asarray(routing_map, bool)
    w1 = np.asarray(w1, np.float32)
    w2 = np.asarray(w2, np.float32)

    x = hidden_states.reshape(T, H)
    res = mlp_residual.reshape(T, H)
    xt_full = np.ascontiguousarray(x.T.astype(MM_NP))  # [H, T]

    n_slots, C, windows = _plan_windows(routing_map, probs)
    # blocked w1 per expert: [NF, 128, H] with [Fc, h, Hc*128+f]
    w1blk = [np.ascontiguousarray(
        w1[e].astype(MM_NP).reshape(NH, 128, NF, 128).transpose(2, 1, 0, 3)
        .reshape(NF, 128, H)) for e in range(E)]
    w2h = w2.astype(MM_NP)

    in_maps = []
    for (experts, lst) in windows:
        n = len(lst)
        tok = np.array([v[0] for v in lst], np.int64)
        xt = np.zeros((H, C), MM_NP)
        if n:
            xt[:, :n] = xt_full[:, tok]
        pp = np.zeros((n_slots, C), np.float32)
        rr = np.zeros((C, H), np.float32)
        if n_slots == 2:
            if n:
                pp[0, :n] = [v[1] for v in lst]
                pp[1, :n] = [v[2] for v in lst]
                first = np.array([v[3] for v in lst], bool)
                rr[:n][first] = res[tok[first]]
        else:  # dense fallback: p = masked probs
            pp[:, :n] = (probs[tok] * routing_map[tok]).T
            rr[:n] = res[tok]
        w1b = np.stack([w1blk[e] for e in experts])
        w2b = np.stack([w2h[e] for e in experts])
        in_maps.append({"xt": xt, "w1b": w1b, "w2b": w2b, "pp": pp,
                        "res": rr})

    nc = _build_nc(n_slots, C)
    r = run_bass_kernel_spmd(nc, in_maps, list(range(N_CORES)),
                             trace=_trace)

    out = np.zeros((T, H), np.float32)
    ids = np.concatenate([[v[0] for v in lst] for (_, lst) in windows
                          if lst]).astype(np.int64)
    rows = np.concatenate([r.results[c]["out"][:len(windows[c][1])]
                           for c in range(N_CORES) if windows[c][1]])
    if len(np.unique(ids)) == len(ids):
        out[ids] = rows
    else:
        np.add.at(out, ids, rows)
    result = out.reshape(S, B, H)
    if _trace:
        return result, r
    return result



# revision 2
# speedup vs baseline: 1.4842x; 1.4842x over previous
"""MoE MLP (E=4, top-2 routing) Trainium2 kernel, 8 NeuronCores.

Strategy (expert x F-half tensor parallel): core c handles expert e = c//2
and F-half = c%2 (columns [half*2048, (half+1)*2048) of w1, matching rows of
w2).  Each core computes, for every token routed to its expert,
    y_half = gelu(x @ w1[e, :, half]) @ w2[e, half, :]
with tokens living in the matmul FREE dimension, so the token count needs no
128-padding -- the compiled program uses C = max_e n_e token columns.  The
host gathers the two halves, scales by the routing probs, scatters back to
token order and adds the residual (no arithmetic on the device's critical
path for any of that).

This covers ANY routing map: every (token, expert) pair with routing_map
True lands in expert e's token list; tokens with 0 experts just pass the
residual through.  Capacity per expert is the full token count (the 2 cores
of an expert both see all its tokens), so no fallback path is needed.
"""
import math
import sys

import numpy as np

try:
    import concourse.bass as bass  # noqa: F401
except Exception:
    sys.path.insert(0, "/opt/trn_rl_repo")

import concourse.bacc as bacc
import concourse.bass as bass
import concourse.mybir as mybir
import concourse.tile as tile
from concourse.bass_utils import run_bass_kernel_spmd

S, B, H, F, E = 1024, 2, 1024, 4096, 4
T = S * B
N_CORES = 8
FH = F // 2     # 2048, per-core F slice
NH = H // 128   # 8 h-chunks
NFH = FH // 128  # 16 f-chunks per core
MM_DT = mybir.dt.float16  # full PE rate, ~2^-11 operand rounding
MM_NP = np.float16

_NC_CACHE = {}


def _pieces(C):
    """Split C token columns into near-equal pieces of <= 512 (PSUM bank)."""
    P = max(1, math.ceil(C / 512))
    base, rem = divmod(C, P)
    widths = [base + 1] * rem + [base] * (P - rem)
    offs, o = [], 0
    for w in widths:
        offs.append(o)
        o += w
    return list(zip(offs, widths))


def _build_nc(C):
    key = (C,)
    if key in _NC_CACHE:
        return _NC_CACHE[key]
    pieces = _pieces(C)
    P = len(pieces)
    f32 = mybir.dt.float32
    gelu = mybir.ActivationFunctionType.Gelu
    nc = bacc.Bacc("TRN2", target_bir_lowering=False, debug=False,
                   num_devices=N_CORES)
    xt_d = nc.declare_dram_parameter("xt", [H, C], MM_DT, isOutput=False)
    w1_d = nc.declare_dram_parameter("w1h", [H, FH], MM_DT, isOutput=False)
    w2_d = nc.declare_dram_parameter("w2h", [FH, H], MM_DT, isOutput=False)
    y_d = nc.declare_dram_parameter("y", [H, C], f32, isOutput=True)

    with tile.TileContext(nc) as tc:
        with (
            tc.tile_pool(name="resident", bufs=1) as rpool,
            tc.tile_pool(name="a", bufs=3) as apool,
            tc.tile_pool(name="y", bufs=2) as ypool,
            tc.tile_pool(name="pa", bufs=2, space="PSUM") as papool,
            tc.tile_pool(name="py", bufs=2, space="PSUM") as pypool,
        ):
            xt_sb = rpool.tile([128, NH, C], MM_DT, tag="xt")
            w1_sb = rpool.tile([128, NH, FH], MM_DT, tag="w1")
            w2_sb = rpool.tile([128, NFH, H], MM_DT, tag="w2")
            xt_ap = xt_d.ap().rearrange("(hc h) c -> h hc c", h=128)
            w1_ap = w1_d.ap().rearrange("(hc h) f -> h hc f", h=128)
            w2_ap = w2_d.ap().rearrange("(fc f) h -> f fc h", f=128)
            y_ap = y_d.ap().rearrange("(hc h) c -> h hc c", h=128)

            # DMA order = need order: x piece0, all w1 (mm1(0)), then the
            # rest of x, then w2 (first needed after mm1(0)+mm1(1)).
            o0, w0 = pieces[0]
            nc.sync.dma_start(xt_sb[:, :, o0:o0 + w0], xt_ap[:, :, o0:o0 + w0])
            for fq in range(NFH):
                nc.sync.dma_start(w1_sb[:, :, fq * 128:(fq + 1) * 128],
                                  w1_ap[:, :, fq * 128:(fq + 1) * 128])
            for (o, w) in pieces[1:]:
                nc.sync.dma_start(xt_sb[:, :, o:o + w], xt_ap[:, :, o:o + w])
            for fq in range(NFH):
                nc.sync.dma_start(w2_sb[:, fq, :], w2_ap[:, fq, :])

            a_tiles = {}

            def mm1(p):
                off, W = pieces[p]
                a_sb = apool.tile([128, NFH, W], MM_DT, tag="a",
                                  name=f"a_{p}")
                a_tiles[p] = a_sb
                for fq in range(NFH):
                    pa = papool.tile([128, W], f32, tag="pa")
                    for hc in range(NH):
                        nc.tensor.matmul(
                            pa[:, :],
                            w1_sb[:, hc, fq * 128:(fq + 1) * 128],
                            xt_sb[:, hc, off:off + W],
                            start=(hc == 0), stop=(hc == NH - 1))
                    nc.scalar.activation(a_sb[:, fq, :], pa[:, :], gelu)

            def mm2(p):
                off, W = pieces[p]
                a_sb = a_tiles.pop(p)
                y_sb = ypool.tile([128, NH, W], f32, tag="y", name=f"y_{p}")
                for hc in range(NH):
                    py = pypool.tile([128, W], f32, tag="py")
                    for fq in range(NFH):
                        nc.tensor.matmul(
                            py[:, :],
                            w2_sb[:, fq, hc * 128:(hc + 1) * 128],
                            a_sb[:, fq, :],
                            start=(fq == 0), stop=(fq == NFH - 1))
                    nc.vector.tensor_scalar_mul(y_sb[:, hc, :], py[:, :], 1.0)
                    nc.sync.dma_start(y_ap[:, hc, off:off + W],
                                      y_sb[:, hc, :])

            # Interleave so piece p's gelus fully overlap PE work, and the
            # PE never waits on the ACT engine at a piece boundary.
            mm1(0)
            for p in range(1, P):
                mm1(p)
                mm2(p - 1)
            mm2(P - 1)
    nc.compile()
    _NC_CACHE[key] = nc
    return nc


def kernel(hidden_states, mlp_residual, probs, routing_map, w1, w2,
           _trace=False):
    hidden_states = np.asarray(hidden_states, np.float32)
    mlp_residual = np.asarray(mlp_residual, np.float32)
    probs = np.asarray(probs, np.float32)
    routing_map = np.asarray(routing_map, bool)
    w1 = np.asarray(w1, np.float32)
    w2 = np.asarray(w2, np.float32)

    x = hidden_states.reshape(T, H)
    idx = [np.nonzero(routing_map[:, e])[0] for e in range(E)]
    C = max(1, max(len(i) for i in idx))

    nc = _build_nc(C)

    in_maps = []
    for c in range(N_CORES):
        e, half = divmod(c, 2)
        tok = idx[e]
        xt = np.zeros((H, C), MM_NP)
        if len(tok):
            xt[:, :len(tok)] = x[tok].T.astype(MM_NP)
        w1h = np.ascontiguousarray(
            w1[e, :, half * FH:(half + 1) * FH].astype(MM_NP))
        w2h = np.ascontiguousarray(
            w2[e, half * FH:(half + 1) * FH, :].astype(MM_NP))
        in_maps.append({"xt": xt, "w1h": w1h, "w2h": w2h})

    r = run_bass_kernel_spmd(nc, in_maps, list(range(N_CORES)), trace=_trace)

    out = mlp_residual.reshape(T, H).astype(np.float32).copy()
    for e in range(E):
        tok = idx[e]
        if len(tok) == 0:
            continue
        y = (np.asarray(r.results[2 * e]["y"][:, :len(tok)], np.float32)
             + np.asarray(r.results[2 * e + 1]["y"][:, :len(tok)],
                          np.float32))
        psel = probs[tok, e].astype(np.float32)
        out[tok] += (y * psel[None, :]).T
    result = out.reshape(S, B, H)
    if _trace:
        return result, r
    return result


# revision 6
# speedup vs baseline: 1.5394x; 1.0372x over previous
"""MoE MLP (E=4, top-2 routing) Trainium2 kernel, 8 NeuronCores.

Strategy (expert x F-half tensor parallel): core c handles expert e = c//2
and F-half = c%2 (columns [half*2048, (half+1)*2048) of w1, matching rows of
w2).  Each core computes, for every token routed to its expert,
    y_half = gelu(x @ w1[e, :, half]) @ w2[e, half, :]
with tokens living in the matmul FREE dimension, so the token count needs no
128-padding -- the compiled program uses C = max_e n_e token columns.  The
host gathers the two halves, scales by the routing probs, scatters back to
token order and adds the residual (no arithmetic on the device's critical
path for any of that).

This covers ANY routing map: every (token, expert) pair with routing_map
True lands in expert e's token list; tokens with 0 experts just pass the
residual through.  Capacity per expert is the full token count (the 2 cores
of an expert both see all its tokens), so no fallback path is needed.
"""
import math
import sys

import numpy as np

try:
    import concourse.bass as bass  # noqa: F401
except Exception:
    sys.path.insert(0, "/opt/trn_rl_repo")

import concourse.bacc as bacc
import concourse.bass as bass
import concourse.mybir as mybir
import concourse.tile as tile
from concourse.bass_utils import run_bass_kernel_spmd

S, B, H, F, E = 1024, 2, 1024, 4096, 4
T = S * B
N_CORES = 8
FH = F // 2     # 2048, per-core F slice
NH = H // 128   # 8 h-chunks
NFH = FH // 128  # 16 f-chunks per core
MM_DT = mybir.dt.float16  # full PE rate, ~2^-11 operand rounding
MM_NP = np.float16

_NC_CACHE = {}


def _pieces(C):
    """Split C token columns into pieces of <= 512 (PSUM bank limit).

    The LAST piece is made small (~18% of C, >= 128) so the post-matmul tail
    (psum copy + output DMA) is short; the other pieces are near-equal.
    """
    if C <= 512:
        widths = [C]
    else:
        last = max(128, min(512, round(C * 0.18)))
        P = max(1, math.ceil((C - last) / 512))
        base, rem = divmod(C - last, P)
        widths = [base + 1] * rem + [base] * (P - rem) + [last]
    offs, o = [], 0
    for w in widths:
        offs.append(o)
        o += w
    return list(zip(offs, widths))


def _build_nc(C):
    key = (C,)
    if key in _NC_CACHE:
        return _NC_CACHE[key]
    pieces = _pieces(C)
    P = len(pieces)
    f32 = mybir.dt.float32
    gelu = mybir.ActivationFunctionType.Gelu
    nc = bacc.Bacc("TRN2", target_bir_lowering=False, debug=False,
                   num_devices=N_CORES)
    xt_d = nc.declare_dram_parameter("xt", [H, C], MM_DT, isOutput=False)
    # w1 host-blocked to [h, fq, hc, f] so each fq chunk is one DMA with 2KB
    # contiguous runs (runs < 512B pay a 2x DMA latency penalty).
    w1_d = nc.declare_dram_parameter("w1b", [128, NFH, NH, 128], MM_DT,
                                     isOutput=False)
    w2_d = nc.declare_dram_parameter("w2h", [FH, H], MM_DT, isOutput=False)
    y_d = nc.declare_dram_parameter("y", [H, C], f32, isOutput=True)

    with tile.TileContext(nc) as tc:
        with (
            tc.tile_pool(name="resident", bufs=1) as rpool,
            tc.tile_pool(name="a", bufs=3) as apool,
            tc.tile_pool(name="y", bufs=2) as ypool,
            tc.tile_pool(name="pa", bufs=4, space="PSUM") as papool,
            tc.tile_pool(name="py", bufs=3, space="PSUM") as pypool,
        ):
            xt_sb = rpool.tile([128, NH, C], MM_DT, tag="xt")
            w1_sb = rpool.tile([128, NFH, NH, 128], MM_DT, tag="w1")
            w2_sb = rpool.tile([128, NFH, H], MM_DT, tag="w2")
            xt_ap = xt_d.ap().rearrange("(hc h) c -> h hc c", h=128)
            w1_ap = w1_d.ap()
            w2_ap = w2_d.ap().rearrange("(fc f) h -> f fc h", f=128)
            y_ap = y_d.ap().rearrange("(hc h) c -> h hc c", h=128)

            # DMA order = need order: w1[fq0] + x piece0 (split by h-half so
            # the first matmul group unblocks early), the rest of w1, the
            # rest of x, then w2 (first needed only after mm1(0)+mm1(1)).
            o0, w0 = pieces[0]
            nc.sync.dma_start(w1_sb[:, 0], w1_ap[:, 0])
            nc.sync.dma_start(xt_sb[:, :4, o0:o0 + w0],
                              xt_ap[:, :4, o0:o0 + w0])
            nc.sync.dma_start(xt_sb[:, 4:, o0:o0 + w0],
                              xt_ap[:, 4:, o0:o0 + w0])
            for fq in range(1, NFH):
                nc.sync.dma_start(w1_sb[:, fq], w1_ap[:, fq])
            for (o, w) in pieces[1:]:
                nc.sync.dma_start(xt_sb[:, :, o:o + w], xt_ap[:, :, o:o + w])
            for fq in range(NFH):
                nc.sync.dma_start(w2_sb[:, fq, :], w2_ap[:, fq, :])

            a_tiles = {}

            def mm1(p):
                off, W = pieces[p]
                a_sb = apool.tile([128, NFH, W], MM_DT, tag="a",
                                  name=f"a_{p}")
                a_tiles[p] = a_sb
                for fq in range(NFH):
                    pa = papool.tile([128, W], f32, tag="pa")
                    for hc in range(NH):
                        nc.tensor.matmul(
                            pa[:, :],
                            w1_sb[:, fq, hc, :],
                            xt_sb[:, hc, off:off + W],
                            start=(hc == 0), stop=(hc == NH - 1))
                    nc.scalar.activation(a_sb[:, fq, :], pa[:, :], gelu)

            def mm2(p):
                off, W = pieces[p]
                a_sb = a_tiles.pop(p)
                y_sb = ypool.tile([128, NH, W], f32, tag="y", name=f"y_{p}")
                for hc in range(NH):
                    py = pypool.tile([128, W], f32, tag="py")
                    for fq in range(NFH):
                        nc.tensor.matmul(
                            py[:, :],
                            w2_sb[:, fq, hc * 128:(hc + 1) * 128],
                            a_sb[:, fq, :],
                            start=(fq == 0), stop=(fq == NFH - 1))
                    nc.vector.tensor_scalar_mul(y_sb[:, hc, :], py[:, :], 1.0)
                    nc.sync.dma_start(y_ap[:, hc, off:off + W],
                                      y_sb[:, hc, :])

            # Interleave so piece p's gelus fully overlap PE work, and the
            # PE never waits on the ACT engine at a piece boundary.
            mm1(0)
            for p in range(1, P):
                mm1(p)
                mm2(p - 1)
            mm2(P - 1)
    nc.compile()
    _NC_CACHE[key] = nc
    return nc


def kernel(hidden_states, mlp_residual, probs, routing_map, w1, w2,
           _trace=False):
    hidden_states = np.asarray(hidden_states, np.float32)
    mlp_residual = np.asarray(mlp_residual, np.float32)
    probs = np.asarray(probs, np.float32)
    routing_map = np.asarray(routing_map, bool)
    w1 = np.asarray(w1, np.float32)
    w2 = np.asarray(w2, np.float32)

    x = hidden_states.reshape(T, H)
    idx = [np.nonzero(routing_map[:, e])[0] for e in range(E)]
    C = max(1, max(len(i) for i in idx))

    nc = _build_nc(C)

    in_maps = []
    for c in range(N_CORES):
        e, half = divmod(c, 2)
        tok = idx[e]
        xt = np.zeros((H, C), MM_NP)
        if len(tok):
            xt[:, :len(tok)] = x[tok].T.astype(MM_NP)
        w1h = w1[e, :, half * FH:(half + 1) * FH].astype(MM_NP)
        w1b = np.ascontiguousarray(
            w1h.reshape(NH, 128, NFH, 128).transpose(1, 2, 0, 3))
        w2h = np.ascontiguousarray(
            w2[e, half * FH:(half + 1) * FH, :].astype(MM_NP))
        in_maps.append({"xt": xt, "w1b": w1b, "w2h": w2h})

    r = run_bass_kernel_spmd(nc, in_maps, list(range(N_CORES)), trace=_trace)

    out = mlp_residual.reshape(T, H).astype(np.float32).copy()
    for e in range(E):
        tok = idx[e]
        if len(tok) == 0:
            continue
        y = (np.asarray(r.results[2 * e]["y"][:, :len(tok)], np.float32)
             + np.asarray(r.results[2 * e + 1]["y"][:, :len(tok)],
                          np.float32))
        psel = probs[tok, e].astype(np.float32)
        out[tok] += (y * psel[None, :]).T
    result = out.reshape(S, B, H)
    if _trace:
        return result, r
    return result


# revision 7
# speedup vs baseline: 1.8288x; 1.1880x over previous
"""MoE MLP (E=4, top-2 routing) Trainium2 kernel, 8 NeuronCores.

Sharding (expert x F-half tensor parallel): core c handles expert e = c//2
and F-half = c%2 (columns [half*2048, (half+1)*2048) of w1, matching rows of
w2).  Each core computes, for every token routed to its expert,
    y_half = gelu(x @ w1[e, :, half]) @ w2[e, half, :]
with tokens living in the matmul FREE dimension, so the token count needs no
128-padding -- the compiled program uses C = max_e n_e token columns.  The
host gathers the two halves, scales by the routing probs, scatters back to
token order and adds the residual.

Precision: matmuls run in fp8e4m3 with MatmulPerfMode.DoubleRow (two
contraction k-tiles per instruction at 0.5 cycles/row).  Weights use
error-compensated quantization: w*32 ~= q8(w*32) + q8(w*32 - q8(w*32)), the
residual folded in as extra DoubleRow accumulation terms, so only the
activation quantization (x and gelu output) contributes first-order error.
Optionally (T1=3) the x residual dx = q8(x - q8(x)) is compensated too
(host-computed, zero extra device passes).  The *32 scaling keeps weight
values out of the fp8 subnormal range; it is undone for free via the gelu
activation scale (mm1) and the PSUM->SBUF copy scale (mm2).

This covers ANY routing map: every (token, expert) pair with routing_map
True lands in expert e's token list; tokens with 0 experts just pass the
residual through.  Capacity per expert is the full token count, so no
fallback path is needed.
"""
import math
import sys

import numpy as np

try:
    import concourse.bass as bass  # noqa: F401
except Exception:
    sys.path.insert(0, "/opt/trn_rl_repo")

import ml_dtypes

import concourse.bacc as bacc
import concourse.bass as bass
import concourse.mybir as mybir
import concourse.tile as tile
from concourse.bass_utils import run_bass_kernel_spmd

S, B, H, F, E = 1024, 2, 1024, 4096, 4
T = S * B
N_CORES = 8
FH = F // 2     # 2048, per-core F slice
NH = H // 128   # 8 h-chunks
NFH = FH // 128  # 16 f-chunks per core
F8_DT = mybir.dt.float8e4
F8_NP = ml_dtypes.float8_e4m3
WSCALE = 32.0   # weight pre-scale, undone on device

# mm1 terms: (w1s8, x8), (dw1s8, x8) [, (w1s8, dx8) if T1 == 3]
# mm2 terms: (w2s8, a8), (dw2s8, a8)
T1 = 3
T2 = 2

_NC_CACHE = {}


def _pieces(C):
    """Split C token columns into pieces of <= 512 (PSUM bank limit).

    First piece exactly 512 when possible (fp8 DMA runs >= 512B avoid the
    2x small-transfer latency penalty); the LAST piece is small (but >= 160
    so matmul exec stays above the 25ns sequencer dispatch) to shorten the
    post-matmul tail.
    """
    if C <= 512:
        widths = [C]
    else:
        last = max(160, min(512, round(C * 0.16)))
        mid = C - 512 - last
        if mid <= 0:
            widths = [512, C - 512] if C - 512 >= 160 else [C - 160, 160]
        else:
            P = math.ceil(mid / 512)
            base, rem = divmod(mid, P)
            widths = [512] + [base + 1] * rem + [base] * (P - rem) + [last]
    offs, o = [], 0
    for w in widths:
        offs.append(o)
        o += w
    return list(zip(offs, widths))


def _build_nc(C):
    key = (C,)
    if key in _NC_CACHE:
        return _NC_CACHE[key]
    pieces = _pieces(C)
    P = len(pieces)
    f32 = mybir.dt.float32
    gelu = mybir.ActivationFunctionType.Gelu
    dr = mybir.MatmulPerfMode.DoubleRow
    nc = bacc.Bacc("TRN2", target_bir_lowering=False, debug=False,
                   num_devices=N_CORES)
    xt_d = nc.declare_dram_parameter("xt8", [H, C], F8_DT, isOutput=False)
    if T1 == 3:
        dxt_d = nc.declare_dram_parameter("dxt8", [H, C], F8_DT,
                                          isOutput=False)
    # weights host-blocked so each (slot, fq) chunk is one DMA with >=1KB
    # contiguous runs (runs < 512B pay a 2x DMA latency penalty).
    w1_d = nc.declare_dram_parameter("w1b", [2, 128, NFH, NH, 128], F8_DT,
                                     isOutput=False)
    w2_d = nc.declare_dram_parameter("w2b", [2, FH, H], F8_DT, isOutput=False)
    y_d = nc.declare_dram_parameter("y", [H, C], f32, isOutput=True)

    with tile.TileContext(nc) as tc:
        with (
            tc.tile_pool(name="resident", bufs=1) as rpool,
            tc.tile_pool(name="a", bufs=3) as apool,
            tc.tile_pool(name="y", bufs=2) as ypool,
            tc.tile_pool(name="pa", bufs=4, space="PSUM") as papool,
            tc.tile_pool(name="py", bufs=3, space="PSUM") as pypool,
        ):
            xt_sb = rpool.tile([128, NH, C], F8_DT, tag="xt")
            w1_sb = rpool.tile([128, 2, NFH, NH, 128], F8_DT, tag="w1")
            w2_sb = rpool.tile([128, 2, NFH, H], F8_DT, tag="w2")
            xt_ap = xt_d.ap().rearrange("(hc h) c -> h hc c", h=128)
            y_ap = y_d.ap().rearrange("(hc h) c -> h hc c", h=128)

            # DMA order = need order: w1 slot0 fq0 + x piece0 (split by
            # h-half so the first matmul group unblocks early), the rest of
            # w1 slot0, w1 slot1, dx, the rest of x, then w2.
            o0, w0 = pieces[0]
            nc.sync.dma_start(w1_sb[:, 0, 0], w1_d[0][:, 0])
            nc.sync.dma_start(xt_sb[:, :4, o0:o0 + w0],
                              xt_ap[:, :4, o0:o0 + w0])
            nc.sync.dma_start(xt_sb[:, 4:, o0:o0 + w0],
                              xt_ap[:, 4:, o0:o0 + w0])
            for fq in range(1, NFH):
                nc.sync.dma_start(w1_sb[:, 0, fq], w1_d[0][:, fq])
            for fq in range(NFH):
                nc.sync.dma_start(w1_sb[:, 1, fq], w1_d[1][:, fq])
            if T1 == 3:
                dxt_sb = rpool.tile([128, NH, C], F8_DT, tag="dxt")
                dxt_ap = dxt_d.ap().rearrange("(hc h) c -> h hc c", h=128)
                nc.sync.dma_start(dxt_sb[:, :, o0:o0 + w0],
                                  dxt_ap[:, :, o0:o0 + w0])
            for (o, w) in pieces[1:]:
                nc.sync.dma_start(xt_sb[:, :, o:o + w], xt_ap[:, :, o:o + w])
                if T1 == 3:
                    nc.sync.dma_start(dxt_sb[:, :, o:o + w],
                                      dxt_ap[:, :, o:o + w])
            for s in range(2):
                w2_ap = w2_d[s].rearrange("(fc f) h -> f fc h", f=128)
                for fq in range(NFH):
                    nc.sync.dma_start(w2_sb[:, s, fq, :], w2_ap[:, fq, :])

            a_tiles = {}

            def mm1(p):
                off, W = pieces[p]
                a_sb = apool.tile([128, NFH, W], F8_DT, tag="a",
                                  name=f"a_{p}")
                a_tiles[p] = a_sb
                terms = [(0, xt_sb), (1, xt_sb)]
                if T1 == 3:
                    terms.append((0, dxt_sb))
                for fq in range(NFH):
                    pa = papool.tile([128, W], f32, tag="pa")
                    n = len(terms) * (NH // 2)
                    i = 0
                    for s, rhs in terms:
                        for j in range(NH // 2):
                            nc.tensor.matmul(
                                pa[:, :],
                                w1_sb[:, s, fq, 2 * j:2 * j + 2, :],
                                rhs[:, 2 * j:2 * j + 2, off:off + W],
                                start=(i == 0), stop=(i == n - 1),
                                perf_mode=dr)
                            i += 1
                    # psum holds 32*(x @ w1half); gelu(psum/32) -> fp8 a
                    nc.scalar.activation(a_sb[:, fq, :], pa[:, :], gelu,
                                         scale=1.0 / WSCALE)

            def mm2(p):
                off, W = pieces[p]
                a_sb = a_tiles.pop(p)
                y_sb = ypool.tile([128, NH, W], f32, tag="y", name=f"y_{p}")
                for hc in range(NH):
                    py = pypool.tile([128, W], f32, tag="py")
                    n = T2 * (NFH // 2)
                    i = 0
                    for s in range(T2):
                        for j in range(NFH // 2):
                            nc.tensor.matmul(
                                py[:, :],
                                w2_sb[:, s, 2 * j:2 * j + 2,
                                      hc * 128:(hc + 1) * 128],
                                a_sb[:, 2 * j:2 * j + 2, :],
                                start=(i == 0), stop=(i == n - 1),
                                perf_mode=dr)
                            i += 1
                    # psum holds 32*(a @ w2half); copy-with-scale undoes it
                    nc.vector.tensor_scalar_mul(y_sb[:, hc, :], py[:, :],
                                                1.0 / WSCALE)
                    nc.sync.dma_start(y_ap[:, hc, off:off + W],
                                      y_sb[:, hc, :])

            # Interleave so piece p's gelus fully overlap PE work, and the
            # PE never waits on the ACT engine at a piece boundary.
            mm1(0)
            for p in range(1, P):
                mm1(p)
                mm2(p - 1)
            mm2(P - 1)
    nc.compile()
    _NC_CACHE[key] = nc
    return nc


def _q8(v):
    return np.asarray(v, F8_NP)


def _block_w1(w):
    """[H, FH] -> [128, NFH, NH, 128] (h, fq, hc, f)."""
    return np.ascontiguousarray(
        w.reshape(NH, 128, NFH, 128).transpose(1, 2, 0, 3))


def kernel(hidden_states, mlp_residual, probs, routing_map, w1, w2,
           _trace=False):
    hidden_states = np.asarray(hidden_states, np.float32)
    mlp_residual = np.asarray(mlp_residual, np.float32)
    probs = np.asarray(probs, np.float32)
    routing_map = np.asarray(routing_map, bool)
    w1 = np.asarray(w1, np.float32)
    w2 = np.asarray(w2, np.float32)

    x = hidden_states.reshape(T, H)
    idx = [np.nonzero(routing_map[:, e])[0] for e in range(E)]
    C = max(1, max(len(i) for i in idx))

    nc = _build_nc(C)

    in_maps = []
    for c in range(N_CORES):
        e, half = divmod(c, 2)
        tok = idx[e]
        xtf = np.zeros((C, H), np.float32)
        if len(tok):
            xtf[:len(tok)] = x[tok]
        x8 = _q8(xtf)
        m = {"xt8": np.ascontiguousarray(x8.astype(np.float32).T).astype(
            F8_NP)}
        if T1 == 3:
            dx8 = _q8(xtf - x8.astype(np.float32))
            m["dxt8"] = np.ascontiguousarray(
                dx8.astype(np.float32).T).astype(F8_NP)
        w1s = w1[e, :, half * FH:(half + 1) * FH] * WSCALE
        w1s8 = _q8(w1s)
        dw1s8 = _q8(w1s - w1s8.astype(np.float32))
        m["w1b"] = np.stack([_block_w1(w1s8.astype(np.float32)),
                             _block_w1(dw1s8.astype(np.float32))]).astype(
            F8_NP)
        w2s = w2[e, half * FH:(half + 1) * FH, :] * WSCALE
        w2s8 = _q8(w2s)
        dw2s8 = _q8(w2s - w2s8.astype(np.float32))
        m["w2b"] = np.stack([w2s8.astype(np.float32),
                             dw2s8.astype(np.float32)]).astype(F8_NP)
        in_maps.append(m)

    r = run_bass_kernel_spmd(nc, in_maps, list(range(N_CORES)), trace=_trace)

    out = mlp_residual.reshape(T, H).astype(np.float32).copy()
    for e in range(E):
        tok = idx[e]
        if len(tok) == 0:
            continue
        y = (np.asarray(r.results[2 * e]["y"][:, :len(tok)], np.float32)
             + np.asarray(r.results[2 * e + 1]["y"][:, :len(tok)],
                          np.float32))
        psel = probs[tok, e].astype(np.float32)
        out[tok] += (y * psel[None, :]).T
    result = out.reshape(S, B, H)
    if _trace:
        return result, r
    return result


# revision 8
# speedup vs baseline: 2.1899x; 1.1975x over previous
"""MoE MLP (E=4, top-2 routing) Trainium2 kernel, 8 NeuronCores.

Sharding (expert x F-half tensor parallel): core c handles expert e = c//2
and F-half = c%2 (columns [half*2048, (half+1)*2048) of w1, matching rows of
w2).  Each core computes, for every token routed to its expert,
    y_half = gelu(x @ w1[e, :, half]) @ w2[e, half, :]
with tokens living in the matmul FREE dimension, so the token count needs no
128-padding -- the compiled program uses C = max_e n_e token columns.  The
host gathers the two halves, scales by the routing probs, scatters back to
token order and adds the residual.

Precision: matmuls run in fp8e4m3 with MatmulPerfMode.DoubleRow (two
contraction k-tiles per instruction at 0.5 cycles/row).  Weights use
error-compensated quantization: w*32 ~= q8(w*32) + q8(w*32 - q8(w*32)), the
residual folded in as extra DoubleRow accumulation terms, so only the
activation quantization (x and gelu output) contributes first-order error.
Optionally (T1=3) the x residual dx = q8(x - q8(x)) is compensated too
(host-computed, zero extra device passes).  The *32 scaling keeps weight
values out of the fp8 subnormal range; it is undone for free via the gelu
activation scale (mm1) and the PSUM->SBUF copy scale (mm2).

This covers ANY routing map: every (token, expert) pair with routing_map
True lands in expert e's token list; tokens with 0 experts just pass the
residual through.  Capacity per expert is the full token count, so no
fallback path is needed.
"""
import math
import sys

import numpy as np

try:
    import concourse.bass as bass  # noqa: F401
except Exception:
    sys.path.insert(0, "/opt/trn_rl_repo")

import ml_dtypes

import concourse.bacc as bacc
import concourse.bass as bass
import concourse.mybir as mybir
import concourse.tile as tile
from concourse.bass_utils import run_bass_kernel_spmd

S, B, H, F, E = 1024, 2, 1024, 4096, 4
T = S * B
N_CORES = 8
FH = F // 2     # 2048, per-core F slice
NH = H // 128   # 8 h-chunks
NFH = FH // 128  # 16 f-chunks per core
F8_DT = mybir.dt.float8e4
F8_NP = ml_dtypes.float8_e4m3
WSCALE = 32.0   # weight pre-scale, undone on device

# mm1 terms: (w1s8, x8), (dw1s8, x8) [, (w1s8, dx8) if T1 == 3]
# mm2 terms: (w2s8, a8), (dw2s8, a8)
T1 = 3
T2 = 2

_NC_CACHE = {}


def _pieces(C):
    """Split C token columns into pieces of <= 512 (PSUM bank limit).

    First piece exactly 512 when possible (fp8 DMA runs >= 512B avoid the
    2x small-transfer latency penalty); the LAST piece is small (but >= 160
    so matmul exec stays above the 25ns sequencer dispatch) to shorten the
    post-matmul tail.
    """
    if C <= 512:
        widths = [C]
    else:
        last = max(160, min(512, round(C * 0.16)))
        mid = C - 512 - last
        if mid <= 0:
            widths = [512, C - 512] if C - 512 >= 160 else [C - 160, 160]
        else:
            P = math.ceil(mid / 512)
            base, rem = divmod(mid, P)
            widths = [512] + [base + 1] * rem + [base] * (P - rem) + [last]
    offs, o = [], 0
    for w in widths:
        offs.append(o)
        o += w
    return list(zip(offs, widths))


def _build_nc(C):
    key = (C,)
    if key in _NC_CACHE:
        return _NC_CACHE[key]
    pieces = _pieces(C)
    P = len(pieces)
    f32 = mybir.dt.float32
    gelu = mybir.ActivationFunctionType.Gelu
    dr = mybir.MatmulPerfMode.DoubleRow
    nc = bacc.Bacc("TRN2", target_bir_lowering=False, debug=False,
                   num_devices=N_CORES)
    xt_d = nc.declare_dram_parameter("xt8", [H, C], F8_DT, isOutput=False)
    if T1 == 3:
        dxt_d = nc.declare_dram_parameter("dxt8", [H, C], F8_DT,
                                          isOutput=False)
    # weights host-blocked so each (slot, fq) chunk is one DMA with >=1KB
    # contiguous runs (runs < 512B pay a 2x DMA latency penalty).
    w1_d = nc.declare_dram_parameter("w1b", [2, 128, NFH, NH, 128], F8_DT,
                                     isOutput=False)
    w2_d = nc.declare_dram_parameter("w2b", [2, FH, H], F8_DT, isOutput=False)
    y_d = nc.declare_dram_parameter("y", [H, C], f32, isOutput=True)

    with tile.TileContext(nc) as tc:
        with (
            tc.tile_pool(name="resident", bufs=1) as rpool,
            tc.tile_pool(name="a", bufs=3) as apool,
            tc.tile_pool(name="y", bufs=2) as ypool,
            tc.tile_pool(name="pa", bufs=4, space="PSUM") as papool,
            tc.tile_pool(name="py", bufs=3, space="PSUM") as pypool,
        ):
            xt_sb = rpool.tile([128, NH, C], F8_DT, tag="xt")
            w1_sb = rpool.tile([128, 2, NFH, NH, 128], F8_DT, tag="w1")
            w2_sb = rpool.tile([128, 2, NFH, H], F8_DT, tag="w2")
            xt_ap = xt_d.ap().rearrange("(hc h) c -> h hc c", h=128)
            y_ap = y_d.ap().rearrange("(hc h) c -> h hc c", h=128)

            # DMA order = need order: everything the fq0 group of mm1(0)
            # touches first (w1 both slots + x piece0 + dx piece0), then the
            # remaining w1 slot pairs per fq, then the rest of x, then w2.
            o0, w0 = pieces[0]
            if T1 == 3:
                dxt_sb = rpool.tile([128, NH, C], F8_DT, tag="dxt")
                dxt_ap = dxt_d.ap().rearrange("(hc h) c -> h hc c", h=128)
            nc.sync.dma_start(w1_sb[:, 0, 0], w1_d[0][:, 0])
            nc.sync.dma_start(xt_sb[:, :4, o0:o0 + w0],
                              xt_ap[:, :4, o0:o0 + w0])
            nc.sync.dma_start(xt_sb[:, 4:, o0:o0 + w0],
                              xt_ap[:, 4:, o0:o0 + w0])
            nc.sync.dma_start(w1_sb[:, 1, 0], w1_d[1][:, 0])
            if T1 == 3:
                nc.sync.dma_start(dxt_sb[:, :, o0:o0 + w0],
                                  dxt_ap[:, :, o0:o0 + w0])
            for fq in range(1, NFH):
                nc.sync.dma_start(w1_sb[:, 0, fq], w1_d[0][:, fq])
                nc.sync.dma_start(w1_sb[:, 1, fq], w1_d[1][:, fq])
            for (o, w) in pieces[1:]:
                nc.sync.dma_start(xt_sb[:, :, o:o + w], xt_ap[:, :, o:o + w])
                if T1 == 3:
                    nc.sync.dma_start(dxt_sb[:, :, o:o + w],
                                      dxt_ap[:, :, o:o + w])
            for s in range(2):
                w2_ap = w2_d[s].rearrange("(fc f) h -> f fc h", f=128)
                for fq in range(NFH):
                    nc.sync.dma_start(w2_sb[:, s, fq, :], w2_ap[:, fq, :])

            a_tiles = {}

            def mm1(p):
                off, W = pieces[p]
                a_sb = apool.tile([128, NFH, W], F8_DT, tag="a",
                                  name=f"a_{p}")
                a_tiles[p] = a_sb
                terms = [(0, xt_sb), (1, xt_sb)]
                if T1 == 3:
                    terms.append((0, dxt_sb))
                for fq in range(NFH):
                    pa = papool.tile([128, W], f32, tag="pa")
                    n = len(terms) * (NH // 2)
                    i = 0
                    for s, rhs in terms:
                        for j in range(NH // 2):
                            nc.tensor.matmul(
                                pa[:, :],
                                w1_sb[:, s, fq, 2 * j:2 * j + 2, :],
                                rhs[:, 2 * j:2 * j + 2, off:off + W],
                                start=(i == 0), stop=(i == n - 1),
                                perf_mode=dr)
                            i += 1
                    # psum holds 32*(x @ w1half); gelu(psum/32) -> fp8 a
                    nc.scalar.activation(a_sb[:, fq, :], pa[:, :], gelu,
                                         scale=1.0 / WSCALE)

            def mm2(p):
                off, W = pieces[p]
                a_sb = a_tiles.pop(p)
                y_sb = ypool.tile([128, NH, W], f32, tag="y", name=f"y_{p}")
                for hc in range(NH):
                    py = pypool.tile([128, W], f32, tag="py")
                    n = T2 * (NFH // 2)
                    i = 0
                    for s in range(T2):
                        for j in range(NFH // 2):
                            nc.tensor.matmul(
                                py[:, :],
                                w2_sb[:, s, 2 * j:2 * j + 2,
                                      hc * 128:(hc + 1) * 128],
                                a_sb[:, 2 * j:2 * j + 2, :],
                                start=(i == 0), stop=(i == n - 1),
                                perf_mode=dr)
                            i += 1
                    # psum holds 32*(a @ w2half); copy-with-scale undoes it
                    nc.vector.tensor_scalar_mul(y_sb[:, hc, :], py[:, :],
                                                1.0 / WSCALE)
                    nc.sync.dma_start(y_ap[:, hc, off:off + W],
                                      y_sb[:, hc, :])

            # Interleave so piece p's gelus fully overlap PE work, and the
            # PE never waits on the ACT engine at a piece boundary.
            mm1(0)
            for p in range(1, P):
                mm1(p)
                mm2(p - 1)
            mm2(P - 1)
    nc.compile()
    _NC_CACHE[key] = nc
    return nc


def _q8(v):
    return np.asarray(v, F8_NP)


def _block_w1(w):
    """[H, FH] -> [128, NFH, NH, 128] (h, fq, hc, f)."""
    return np.ascontiguousarray(
        w.reshape(NH, 128, NFH, 128).transpose(1, 2, 0, 3))


def kernel(hidden_states, mlp_residual, probs, routing_map, w1, w2,
           _trace=False):
    hidden_states = np.asarray(hidden_states, np.float32)
    mlp_residual = np.asarray(mlp_residual, np.float32)
    probs = np.asarray(probs, np.float32)
    routing_map = np.asarray(routing_map, bool)
    w1 = np.asarray(w1, np.float32)
    w2 = np.asarray(w2, np.float32)

    x = hidden_states.reshape(T, H)
    idx = [np.nonzero(routing_map[:, e])[0] for e in range(E)]
    C = max(1, max(len(i) for i in idx))

    nc = _build_nc(C)

    in_maps = []
    for c in range(N_CORES):
        e, half = divmod(c, 2)
        tok = idx[e]
        xtf = np.zeros((C, H), np.float32)
        if len(tok):
            xtf[:len(tok)] = x[tok]
        x8 = _q8(xtf)
        m = {"xt8": np.ascontiguousarray(x8.astype(np.float32).T).astype(
            F8_NP)}
        if T1 == 3:
            dx8 = _q8(xtf - x8.astype(np.float32))
            m["dxt8"] = np.ascontiguousarray(
                dx8.astype(np.float32).T).astype(F8_NP)
        w1s = w1[e, :, half * FH:(half + 1) * FH] * WSCALE
        w1s8 = _q8(w1s)
        dw1s8 = _q8(w1s - w1s8.astype(np.float32))
        m["w1b"] = np.stack([_block_w1(w1s8.astype(np.float32)),
                             _block_w1(dw1s8.astype(np.float32))]).astype(
            F8_NP)
        w2s = w2[e, half * FH:(half + 1) * FH, :] * WSCALE
        w2s8 = _q8(w2s)
        dw2s8 = _q8(w2s - w2s8.astype(np.float32))
        m["w2b"] = np.stack([w2s8.astype(np.float32),
                             dw2s8.astype(np.float32)]).astype(F8_NP)
        in_maps.append(m)

    r = run_bass_kernel_spmd(nc, in_maps, list(range(N_CORES)), trace=_trace)

    out = mlp_residual.reshape(T, H).astype(np.float32).copy()
    for e in range(E):
        tok = idx[e]
        if len(tok) == 0:
            continue
        y = (np.asarray(r.results[2 * e]["y"][:, :len(tok)], np.float32)
             + np.asarray(r.results[2 * e + 1]["y"][:, :len(tok)],
                          np.float32))
        psel = probs[tok, e].astype(np.float32)
        out[tok] += (y * psel[None, :]).T
    result = out.reshape(S, B, H)
    if _trace:
        return result, r
    return result
